# revision 18
# baseline (speedup 1.0000x reference)
"""Trainium2 Bass kernel for nn_NumAttention (sparse_attention).

Reference computation (per batch b, head i):
    k     = blockmix(x_cat, softmax(W_K)[i])            # [P, DH]
    xq    = blockmix(x_cat, softmax(W_Q)[i])            # [P, DH]
    q     = xq @ softmax(W_pred)[i]                     # [P, DH]
    v     = x_num @ softmax(W_V)[i]                     # [P]
    z[qp] = sum_{p<=qp} v[p] * (k[p] . q[qp])           # causal, no softmax

Linear attention: z[qp] = xq[qp] . (pp @ S[qp]), S = cumsum_p(v[p] * k[p]).
O(P^2) scores never exist.  Unlike the previous folded-ktilde design, pp is
NOT folded into the mix weight (which made it dense [512 -> 256]); instead:

  * Mix uses PLAIN block-diagonal k (like q): with h-major features each
    128-feature K-tile feeds 64 k-cols + 64 q-cols, and the combined [k|q]
    weight tile [128, 128] is IDENTICAL for all K-tiles and chunks (pk/pq
    have no h dependence).  One matmul per (chunk, ktile): 4096 mix PE
    columns per core instead of 20480.
  * The prefix S is built TRANSPOSED: ST[f, r] = sum_p vk[p,f] triu[p,r]
    with the vk slice as stationary and triu moving (4 slices per pair).
    Chunk-level carries (texw) inject via 2 strided matmuls (stationary =
    [2, 128] texw rows, moving = a [2, 256] delta-selector).
  * pp applies at chunk level: Stilde = ST^T @ blockdiag(ppT) with the
    ST SBUF copy as stationary (M = r, LDW fully hidden) and a [128, 128]
    blockdiag(ppT) wall constant moving.  Output is r-major in PSUM and
    feeds the unchanged q*S multiply + 64-wide reduce.
  * The reduce runs on GpSimd to unload DVE.

16 chunks run as four uneven quarters (6/4/4/2) with a sequential
inter-quarter carry; pass-2 of quarter Q overlaps the mix of Q+1, and the
tail quarter skips the prefix pipeline (its TexL IS the carry; the odd
chunk's total arrives as a ones-matrix broadcast matmul straight into PSUM).

Sharding: 8 cores = 4 batches x 2 head-groups (4 heads each).  Host ships
x_cat[b] transposed h-major bf16 and host-computed v (x_num @ pv, 8 MFLOP).
A burst of skinny dummy matmuls warms the PE HAM clock gate during the DMA
head.
"""

import numpy as np
import ml_dtypes

import concourse.bacc as bacc
import concourse.mybir as mybir
import concourse.tile as tile
from concourse.bass_utils import run_bass_kernel_spmd

B, P, DC, DN, H, DH = 4, 2048, 512, 64, 8, 64
NV = DC // DH     # 8 variables
CH = 128          # positions per chunk
NCH = P // CH     # 16 chunks
HPC = 4           # heads per core
NT = 4            # h-groups of 16 (= KC)
GL = 16           # h's per group
FH = HPC * DH     # 256 = k/q width per core, cols ordered (i, t, glo)
FH2 = 2 * FH      # 512 = chunk-pair width
KC = DC // CH     # 4 feature K-tiles
NQ = 4            # quarters (uneven: big early, tiny last for a short tail)
CPQS = (6, 4, 4, 2)            # chunks per quarter
NPQS = (3, 2, 2, 1)            # pairs per quarter
CB = (0, 6, 10, 14)            # chunk base per quarter
PB = (0, 3, 5, 7)              # pair base per quarter
OH_OFF = (0, 36, 52, 68)       # oneh col offset per quarter (cpq^2 widths)
OHW = 72
SEL0 = 0                       # delta-selector [2, 256] in cs
ST_OFF = 2 * CH                # strt2 blocks offset in cs (interleaved L,R per j)
ST_OFFS = (0, 6, 10, 14)       # per-quarter 2*npq strt col offsets
ONES0 = ST_OFF + 16            # all-ones block [6, 3]
CSW = ONES0 + 3
NPAIR = NCH // 2  # 8 chunk pairs
NCORES = 8
NWARM = 33

# wall (packed weights+consts) column offsets
MIXW0 = 0          # 128: combined [k|q] mix tile, cols (h2, i4, glo16)
BD0 = CH           # 128: 2 x blockdiag(ppT) tiles [128, 128]
V0 = BD0 + 2 * CH  # 384: v [128, 64] cols (c, i)
TR0 = V0 + 64      # 448: triu [128, 128]
OH0 = TR0 + CH     # 576: oneh selectors [128, 72]
OC0 = OH0 + OHW    # 648: all-ones [128, 128]
WALLC = OC0 + CH   # 776

_BF16 = ml_dtypes.bfloat16

_cache = {}


def _softmax(x, axis=-1):
    e = np.exp(x - x.max(axis=axis, keepdims=True))
    return e / e.sum(axis=axis, keepdims=True)


def _build_program():
    nc = bacc.Bacc()
    f32 = mybir.dt.float32
    bf16 = mybir.dt.bfloat16
    mult = mybir.AluOpType.mult
    add = mybir.AluOpType.add

    wall1_d = nc.dram_tensor("wall1", [CH, V0], bf16, kind="ExternalInput")
    wall2_d = nc.dram_tensor("wall2", [CH, WALLC - V0], bf16, kind="ExternalInput")
    xct_d = nc.dram_tensor("xct", [8, CH, KC, P // 8], bf16, kind="ExternalInput")
    cs_d = nc.dram_tensor("cs", [6, CSW], bf16, kind="ExternalInput")
    z_d = nc.dram_tensor("z", [CH, NCH * HPC], f32, kind="ExternalOutput")

    with tile.TileContext(nc) as tc:
        with (
            tc.tile_pool(name="persist", bufs=1) as pers,
            tc.tile_pool(name="work", bufs=3) as work,
            tc.tile_pool(name="pref", bufs=2) as pref,
            tc.tile_pool(name="mix", bufs=2, space="PSUM") as mix,
            tc.tile_pool(name="t2p", bufs=1, space="PSUM") as t2p,
            tc.tile_pool(name="stp", bufs=3, space="PSUM") as stp,
        ):
            wall_sb = pers.tile([CH, WALLC], bf16, tag="wall_sb")
            cs_sb = pers.tile([6, CSW], bf16, tag="cs_sb")
            xcT = pers.tile([CH, 8, KC, P // 8], bf16, tag="xcT")
            vk_sb = pers.tile([CH, NCH, FH], bf16, tag="vk_sb")
            q_sb = pers.tile([CH, NPAIR, FH2], bf16, tag="q_sb")
            z_sb = pers.tile([CH, NCH * HPC], f32, tag="z_sb")
            dumw = pers.tile([CH, CH], bf16, tag="dumw")

            trit = wall_sb[:, TR0 : TR0 + CH]
            mixw = wall_sb[:, MIXW0 : MIXW0 + CH]
            ones_row = wall_sb[0:1, OC0 : OC0 + CH]
            ones11 = cs_sb[0:1, ONES0 : ONES0 + 1]
            ones12 = cs_sb[0:1, ONES0 : ONES0 + 2]
            sel2 = cs_sb[0:2, SEL0 : SEL0 + 2 * CH]

            # ---- PE warm-up: release the HAM clock gate during the DMA head
            nc.vector.memset(dumw[:], 0.0)
            warmps = stp.tile([4, FH2], f32, tag="pST")
            for i in range(NWARM):
                nc.tensor.matmul(
                    warmps[:, 0:CH], dumw[:, 0:4], dumw[:], start=True, stop=True
                )

            # ---- loads: scalar ring carries the one packed weight DMA;
            # sync ring streams x pair-slices (cs squeezed in after xct3).
            nc.sync.dma_start(out=xcT[:, 0], in_=xct_d[0])
            nc.scalar.dma_start(out=wall_sb[:, 0:V0], in_=wall1_d[:])
            nc.scalar.dma_start(out=wall_sb[:, V0:WALLC], in_=wall2_d[:])
            for s in range(1, 8):
                nc.sync.dma_start(out=xcT[:, s], in_=xct_d[s])
                if s == 3:
                    nc.sync.dma_start(out=cs_sb[:], in_=cs_d[:])

            carry_prev = None
            pending_pass2 = None

            def make_pass2(Q):
                npq = NPQS[Q]
                texw_q = texw_tiles.get(Q)
                carry_tail = carry_prev

                def emit():
                    for j in range(npq):
                        Jg = PB[Q] + j
                        pST = stp.tile([CH, FH2], f32, tag="pST")
                        lastmm = Q == NQ - 1
                        for ci in range(2):
                            for hp in range(2):
                                m = ci * 2 + hp
                                nc.tensor.matmul(
                                    pST[:, m * CH : (m + 1) * CH],
                                    vk_sb[:, 2 * Jg + ci, hp * CH : (hp + 1) * CH],
                                    trit,
                                    start=(m == 0),
                                    stop=False,
                                    skip_group_check=True,
                                )
                        if lastmm:
                            # tail: TexL (= carry) to both chunks; the odd
                            # chunk's own total lands as an all-ones matmul
                            for hp in range(2):
                                nc.tensor.matmul(
                                    pST[:, (2 + hp) * CH : (3 + hp) * CH],
                                    vk_sb[:, 2 * Jg, hp * CH : (hp + 1) * CH],
                                    wall_sb[:, OC0 : OC0 + CH],
                                    start=False,
                                    stop=False,
                                    skip_group_check=True,
                                )
                            for ci in range(2):
                                for hp in range(2):
                                    m = ci * 2 + hp
                                    nc.tensor.matmul(
                                        pST[:, m * CH : (m + 1) * CH],
                                        carry_tail[0:1, hp * CH : (hp + 1) * CH],
                                        ones_row,
                                        start=False,
                                        stop=(m == 3),
                                        skip_group_check=True,
                                    )
                        else:
                            pSTv = pST[:].rearrange(
                                "p (c hp r) -> p c hp r", c=2, hp=2
                            )
                            for hp in range(2):
                                nc.tensor.matmul(
                                    pSTv[:, :, hp, :],
                                    texw_q[0:2, (2 * j + hp) * CH : (2 * j + hp + 1) * CH],
                                    sel2,
                                    start=False,
                                    stop=(hp == 1),
                                    skip_group_check=True,
                                )
                        st = work.tile([CH, FH2], bf16, tag="st_sb")
                        nc.scalar.copy(st[:], pST[:])
                        # Stilde overwrites the same PSUM bank (freed by the
                        # st copy) — no extra bank, 3-deep pass2 pipeline
                        for m in range(4):
                            hp = m % 2
                            nc.tensor.matmul(
                                pST[:, m * CH : (m + 1) * CH],
                                st[:, m * CH : (m + 1) * CH],
                                wall_sb[:, BD0 + hp * CH : BD0 + (hp + 1) * CH],
                                start=(m == 0),
                                stop=(m == 3),
                                skip_group_check=True,
                            )
                        prod = work.tile([CH, FH2], bf16, tag="prod")
                        nc.vector.tensor_tensor(
                            out=prod[:], in0=q_sb[:, Jg, :], in1=pST[:], op=mult
                        )
                        # cols are (c, i, g): reduce contiguous g=64.  bf16
                        # out keeps DVE in 2B fast mode; ~0.4% rounding is
                        # well inside the tolerance.
                        nc.vector.tensor_reduce(
                            out=z_sb[:, 2 * Jg * HPC : (2 * Jg + 2) * HPC],
                            in_=prod[:].rearrange("p (a x) -> p a x", x=NT * GL),
                            axis=mybir.AxisListType.X,
                            op=add,
                        )
                    zo = CB[Q] * HPC
                    zw = CPQS[Q] * HPC
                    nc.sync.dma_start(
                        out=z_d[:, zo : zo + zw], in_=z_sb[:, zo : zo + zw]
                    )

                return emit

            texw_tiles = {}
            for Q in range(NQ):
                cpq, npq = CPQS[Q], NPQS[Q]
                lastq = Q == NQ - 1
                t2ps = None if lastq else t2p.tile([6, FH2], f32, tag="t2ps")
                for j in range(npq):  # pairs within quarter
                    Jg = PB[Q] + j
                    ct = mix.tile([CH, 2 * FH2], f32, tag="ct")
                    ctv = ct[:].rearrange(
                        "p (c h i t g) -> p c h i t g", c=2, h=2, i=HPC, t=NT
                    )
                    for cl2 in range(2):
                        c = 2 * Jg + cl2
                        s, off = c // 2, (c % 2) * CH
                        for kc in range(KC):
                            nc.tensor.matmul(
                                ctv[:, cl2, :, :, kc, :],
                                xcT[:, s, kc, off : off + CH],
                                mixw,
                                start=(kc == 0),
                                stop=(kc == KC - 1),
                                skip_group_check=True,
                            )
                    # vk = k * v  (one TT per pair; v bcast over (t,g))
                    nc.vector.tensor_tensor(
                        out=vk_sb[:, 2 * Jg : 2 * Jg + 2, :].rearrange(
                            "p c (i x) -> p c i x", i=HPC
                        ),
                        in0=ctv[:, :, 0].rearrange("p c i t g -> p c i (t g)"),
                        in1=wall_sb[:, V0 + 2 * Jg * HPC : V0 + (2 * Jg + 2) * HPC]
                        .rearrange("p (c i) -> p c i", c=2)
                        .unsqueeze(3)
                        .broadcast_to([CH, 2, HPC, NT * GL]),
                        op=mult,
                    )
                    # split the q drain across ACT and DVE
                    nc.scalar.copy(q_sb[:, Jg, 0:FH], ct[:, FH:FH2])
                    nc.vector.tensor_copy(q_sb[:, Jg, FH:FH2], ct[:, FH2 + FH :])
                    # per-chunk column sums (not needed for the tail quarter)
                    for cl2 in range(2) if not lastq else ():
                        cl = 2 * j + cl2
                        nc.tensor.matmul(
                            t2ps[0:cpq, 0:FH],
                            wall_sb[
                                :, OH0 + OH_OFF[Q] + cl * cpq : OH0
                                + OH_OFF[Q] + (cl + 1) * cpq
                            ],
                            vk_sb[:, CB[Q] + cl, :],
                            start=(cl == 0),
                            stop=(cl == cpq - 1),
                        )

                # ---- prefix for this quarter (chunk-granular cumsums).
                # The tail quarter (1 pair) needs none of it: its TexL IS
                # carry_prev and TexR arrives as a PE broadcast-matmul.
                if lastq:
                    if pending_pass2 is not None:
                        pending_pass2()
                    pending_pass2 = make_pass2(Q)
                    continue
                t2q_sb = pref.tile([6, FH], bf16, tag="t2q_sb")
                nc.scalar.copy(t2q_sb[0:cpq, :], t2ps[0:cpq, 0:FH])
                soff = ST_OFF + ST_OFFS[Q]
                texw_sb = pref.tile([2, 6 * CH], bf16, tag="texw_sb")
                for j in range(npq):
                    tps = stp.tile([2, 2 * CH], f32, tag="pST")
                    last = carry_prev is None
                    strt2 = cs_sb[0:cpq, soff + 2 * j : soff + 2 * j + 2]
                    for hp in range(2):
                        nc.tensor.matmul(
                            tps[0:2, hp * CH : (hp + 1) * CH],
                            strt2,
                            t2q_sb[0:cpq, hp * CH : (hp + 1) * CH],
                            start=True,
                            stop=last,
                            skip_group_check=True,
                        )
                        if carry_prev is not None:
                            nc.tensor.matmul(
                                tps[0:2, hp * CH : (hp + 1) * CH],
                                ones12,
                                carry_prev[0:1, hp * CH : (hp + 1) * CH],
                                start=False,
                                stop=True,
                                skip_group_check=True,
                            )
                    nc.scalar.copy(
                        texw_sb[0:2, 2 * j * CH : (2 * j + 2) * CH], tps[0:2, :]
                    )
                texw_tiles[Q] = texw_sb
                if Q < NQ - 1:
                    # carry accumulates in the t2 bank (free region)
                    ones_n1 = cs_sb[0:cpq, ONES0 : ONES0 + 1]
                    last = carry_prev is None
                    nc.tensor.matmul(
                        t2ps[0:1, FH:FH2], ones_n1, t2q_sb[0:cpq, :], start=True,
                        stop=last,
                    )
                    if carry_prev is not None:
                        nc.tensor.matmul(
                            t2ps[0:1, FH:FH2],
                            ones11,
                            carry_prev[:],
                            start=False,
                            stop=True,
                        )
                    carry_new = pref.tile([1, FH], bf16, tag="carry_sb")
                    nc.scalar.copy(carry_new[:], t2ps[0:1, FH:FH2])
                    carry_prev = carry_new

                # emit the PREVIOUS quarter's pass 2 now: its prefix chain has
                # settled, and the current quarter's chain ops keep priority
                if pending_pass2 is not None:
                    pending_pass2()
                pending_pass2 = make_pass2(Q)

            pending_pass2()  # last quarter's pass 2

    nc.finalize()
    return nc


def _host_inputs(x_cat, x_num, W_K, W_Q, W_pred, W_V):
    """Per-core input maps. Core c = batch (c//2), head-group (c%2)."""
    pk = _softmax(W_K.astype(np.float64)).astype(np.float32)
    pq = _softmax(W_Q.astype(np.float64)).astype(np.float32)
    pp = _softmax(W_pred.astype(np.float64)).astype(np.float32)
    pv = _softmax(W_V.astype(np.float64)).astype(np.float32)

    v_full = np.einsum("bpd,id->bpi", x_num, pv)  # [B, P, H] fp32

    # constants (cs [6, CSW]): delta-selector rows, per-quarter strt2
    # blocks (interleaved L,R per pair), all-ones block
    cs = np.zeros((6, CSW), np.float32)
    cs[0, SEL0 : SEL0 + CH] = 1.0
    cs[1, SEL0 + CH : SEL0 + 2 * CH] = 1.0
    for Q in range(NQ):
        cpq, npq = CPQS[Q], NPQS[Q]
        soff = ST_OFF + ST_OFFS[Q]
        for k in range(cpq):
            for m in range(npq):
                cs[k, soff + 2 * m] = 1.0 if k < 2 * m else 0.0
                cs[k, soff + 2 * m + 1] = 1.0 if k <= 2 * m else 0.0
    cs[:, ONES0 : ONES0 + 3] = 1.0

    trit = np.triu(np.ones((CH, CH), np.float32))
    oneh = np.zeros((CH, OHW), np.float32)
    for Q in range(NQ):
        cpq = CPQS[Q]
        for cl in range(cpq):
            oneh[:, OH_OFF[Q] + cl * cpq + cl] = 1.0

    in_maps = []
    for core in range(NCORES):
        b, hg = core // 2, core % 2
        heads = [hg * HPC + j for j in range(HPC)]
        # h-major features: f' = h*NV + v
        x_hm = x_cat[b].reshape(P, NV, DH).transpose(0, 2, 1).reshape(P, DC)
        xct = np.ascontiguousarray(
            x_hm.T.reshape(KC, CH, 8, P // 8).transpose(2, 1, 0, 3)
        ).astype(_BF16)

        # combined [k|q] mix tile [128, 128]: rows f = (glo_f 16, v 8),
        # cols (h2, i4, glo16); identical for every K-tile and chunk.
        mixw = np.zeros((CH, CH), np.float32)
        for il, hd in enumerate(heads):
            for g in range(GL):
                mixw[g * NV : (g + 1) * NV, 0 * 64 + il * GL + g] = pk[hd]
                mixw[g * NV : (g + 1) * NV, 1 * 64 + il * GL + g] = pq[hd]

        # blockdiag(ppT) tiles [128, 128] x 2 head-pairs
        bds = np.zeros((2, CH, CH), np.float32)
        for hp in range(2):
            for i2 in range(2):
                hd = heads[2 * hp + i2]
                bds[hp, i2 * DH : (i2 + 1) * DH, i2 * DH : (i2 + 1) * DH] = pp[hd].T

        # v [128, (c, i)]
        v_core = v_full[b][:, heads]  # [P, 4]
        v_dev = v_core.reshape(NCH, CH, HPC).transpose(1, 0, 2).reshape(CH, NCH * HPC)

        wall1 = np.concatenate([mixw, bds[0], bds[1]], axis=1)   # [128, 384]
        wall2 = np.concatenate(
            [v_dev, trit, oneh, np.ones((CH, CH), np.float32)], axis=1
        )  # [128, 392]

        in_maps.append(
            {
                "xct": xct,
                "wall1": np.ascontiguousarray(wall1).astype(_BF16),
                "wall2": np.ascontiguousarray(wall2).astype(_BF16),
                "cs": cs.astype(_BF16),
            }
        )
    return in_maps


def _run(inputs, **spmd_kwargs):
    if "nc" not in _cache:
        _cache["nc"] = _build_program()
    nc = _cache["nc"]

    in_maps = _host_inputs(**inputs)
    res = run_bass_kernel_spmd(nc, in_maps, list(range(NCORES)), **spmd_kwargs)

    out = np.zeros((B, P, H), np.float32)
    for core in range(NCORES):
        b, hg = core // 2, core % 2
        z = np.asarray(res.results[core]["z"], np.float32)  # [128, NCH*HPC]
        z = z.reshape(CH, NCH, HPC).transpose(1, 0, 2).reshape(P, HPC)
        out[b, :, hg * HPC : (hg + 1) * HPC] = z
    return out, res


def kernel(x_cat, x_num, W_K, W_Q, W_pred, W_V):
    out, _ = _run(
        dict(x_cat=x_cat, x_num=x_num, W_K=W_K, W_Q=W_Q, W_pred=W_pred, W_V=W_V)
    )
    return out
